# revision 10
# baseline (speedup 1.0000x reference)
"""ComplexLSTM Trainium2 kernel.

Problem: B=32, I=128, H=256, T=2048. Four independent LSTM scans
(real/imag weights x real/imag inputs) combined into a complex output
(B, H, T) complex64.

Sharding: data-parallel over batch across 8 cores (4 rows each); each
core runs all four scans for its batch slice, organized as two "chains"
that share a recurrent weight matrix (Whh_r / Whh_i), so each chain's
per-step recurrent matmul streams its weight once for both scans.

Per-step math (PyTorch gate order i, f, g, o; g rows pre-scaled by 2 on
host so a single sigmoid covers all gates: tanh(x) = 2*sigmoid(2x)-1):
  gates = gx_t + h @ Whh.T        (PE; fp32r; gx & bias folded in)
  S     = sigmoid(gates)          (ACT, one instruction)
  c'    = S_f*c + 2*S_i*S_g - S_i (DVE)
  h     = 2*S_o*sigmoid(2c') - S_o (ACT + DVE)
h is transposed each step on the PE (identity-matmul transpose) to feed
the next step's stationary operand.
"""

import numpy as np
from contextlib import ExitStack

import concourse.bass as bass
import concourse.tile as tile
import concourse.mybir as mybir
from concourse import bacc
from concourse.bass import ds
from concourse.bass_utils import run_bass_kernel_spmd
from concourse.masks import make_identity

B, I, H = 32, 128, 256
G = 4 * H            # 1024
NCORES = 8
BL = B // NCORES     # 4 batch rows per core
ROWS = 2 * BL        # 8 rows per chain (2 scans x 4 batch)
U = 8                # steps per For_i iteration

f32 = mybir.dt.float32
f32r = mybir.dt.float32r
SIG = mybir.ActivationFunctionType.Sigmoid
MULT = mybir.AluOpType.mult
SUB = mybir.AluOpType.subtract


def r(ap):
    return ap.bitcast(f32r)


def build_program(T):
    TC = T // 128      # phase-1 tiles per (scan, b)
    nc = bacc.Bacc("TRN2", target_bir_lowering=False, debug=False,
                   num_devices=NCORES)

    xr = nc.declare_dram_parameter("xr", [BL, I, T], f32r, isOutput=False)
    xi = nc.declare_dram_parameter("xi", [BL, I, T], f32r, isOutput=False)
    wih = nc.declare_dram_parameter("wih", [2, I, G], f32r, isOutput=False)
    whh = nc.declare_dram_parameter("whh", [2, H, G], f32r, isOutput=False)
    bia = nc.declare_dram_parameter("bias", [2, G], f32, isOutput=False)
    sgn = nc.declare_dram_parameter("sgn", [ROWS, 1], f32, isOutput=False)
    id8 = nc.declare_dram_parameter("id8", [ROWS, ROWS], f32r, isOutput=False)
    out = nc.declare_dram_parameter("out", [BL, H, 2 * T], f32, isOutput=True)

    # staging: gx per chain [T, ROWS, G]; h combine output [T, ROWS, H]
    gxst = [nc.dram_tensor(f"gx_stage{c}", [T, ROWS, G], f32r) for c in range(2)]
    hst = nc.dram_tensor("h_stage", [T, ROWS, H], f32)

    # chain c, slot s -> input tensor (chain0=Wr: xr,xi ; chain1=Wi: xi,xr)
    def xsrc(c, s):
        return (xr if s == 0 else xi) if c == 0 else (xi if s == 0 else xr)

    with tile.TileContext(nc) as tc, ExitStack() as top:
        consts = top.enter_context(tc.tile_pool(name="consts", bufs=1))

        whh_sb = [[consts.tile([128, G], f32r, name=f"whh{c}{k}", tag=f"whh{c}{k}") for k in range(2)]
                  for c in range(2)]
        for c in range(2):
            for k in range(2):
                nc.sync.dma_start(out=whh_sb[c][k],
                                  in_=whh[c, k * 128:(k + 1) * 128, :])

        ident8 = consts.tile([ROWS, ROWS], f32r, tag="ident8")
        nc.sync.dma_start(out=ident8, in_=id8[:, :])
        ident8f = consts.tile([ROWS, ROWS], f32, tag="ident8f")
        make_identity(nc, ident8f)
        sgn_sb = consts.tile([ROWS, 1], f32, tag="sgn_sb")
        nc.sync.dma_start(out=sgn_sb, in_=sgn[:, :])

        # ---------------- phase 1: gx = x @ WihT (+bias) ----------------
        with ExitStack() as p1:
            p1c = p1.enter_context(tc.tile_pool(name="p1c", bufs=1))
            xp = p1.enter_context(tc.tile_pool(name="xp", bufs=4))
            gp = p1.enter_context(tc.tile_pool(name="gp", bufs=2, space="PSUM"))
            gs = p1.enter_context(tc.tile_pool(name="gs", bufs=4))

            wih_sb = [p1c.tile([I, G], f32r, name=f"wih{c}", tag=f"wih{c}") for c in range(2)]
            bia_sb = [p1c.tile([128, G], f32, name=f"bia{c}", tag=f"bia{c}") for c in range(2)]
            for c in range(2):
                nc.sync.dma_start(out=wih_sb[c], in_=wih[c])
                bsrc = bia[c:c + 1, :]
                nc.sync.dma_start(
                    out=bia_sb[c],
                    in_=bass.AP(tensor=bsrc.tensor, offset=bsrc.offset,
                                ap=[[0, 128]] + list(bsrc.ap[-1:])))

            for c in range(2):
                for s in range(2):
                    src = xsrc(c, s)
                    for b in range(BL):
                        for t in range(TC):
                            xt = xp.tile([I, 128], f32r, tag="xt")
                            nc.sync.dma_start(
                                out=xt, in_=src[b, :, t * 128:(t + 1) * 128])
                            ps = gp.tile([128, G], f32, tag="ps")
                            for n in range(2):
                                sl = ds(n * 512, 512)
                                nc.tensor.matmul(ps[:, sl], xt,
                                                 wih_sb[c][:, sl],
                                                 start=True, stop=True)
                            gt = gs.tile([128, G], f32r, tag="gt")
                            nc.vector.tensor_add(
                                gt, ps, bia_sb[c])
                            nc.sync.dma_start(
                                out=gxst[c][t * 128:(t + 1) * 128,
                                            s * BL + b, :],
                                in_=gt.rearrange("p (o g) -> p o g", o=1))

        # ---------------- phase 2: the recurrence ----------------
        with ExitStack() as p2:
            st8 = p2.enter_context(tc.tile_pool(name="st8", bufs=1))
            gxp = p2.enter_context(tc.tile_pool(name="gxp", bufs=2))
            spl = p2.enter_context(tc.tile_pool(name="spl", bufs=2))
            tmp = p2.enter_context(tc.tile_pool(name="tmp", bufs=2))
            hpl = p2.enter_context(tc.tile_pool(name="hpl", bufs=3))
            stg = p2.enter_context(tc.tile_pool(name="stg", bufs=2))
            psA = p2.enter_context(tc.tile_pool(name="psA", bufs=2, space="PSUM"))
            psB = p2.enter_context(tc.tile_pool(name="psB", bufs=1, space="PSUM"))
            psT = p2.enter_context(tc.tile_pool(name="psT", bufs=2, space="PSUM"))

            # persistent state (ping-pong on step parity)
            hT = [[[st8.tile([128, ROWS], f32r, name=f"hT{c}{p}{k}", tag=f"hT{c}{p}{k}")
                    for k in range(2)] for p in range(2)] for c in range(2)]
            cst = [[st8.tile([ROWS, H], f32, name=f"c{c}{p}", tag=f"c{c}{p}")
                    for p in range(2)] for c in range(2)]
            for c in range(2):
                for p in range(2):
                    nc.vector.memset(cst[c][p], 0.0)
                    for k in range(2):
                        nc.vector.memset(hT[c][p][k].bitcast(f32), 0.0)

            with tc.For_i(0, T, U) as iv:
                gxch = [gxp.tile([ROWS, U, G], f32r, name=f"gx{c}", tag=f"gx{c}")
                        for c in range(2)]
                for c in range(2):
                    nc.sync.dma_start(
                        out=gxch[c],
                        in_=gxst[c][ds(iv, U), :, :].rearrange("u p g -> p u g"))
                st = stg.tile([ROWS, U, H], f32, tag="st")
                for k in range(U):
                    pp = k % 2
                    hcur = []
                    for c in range(2):
                        gpool = psA if c == 0 else psB
                        gates = gpool.tile([ROWS, G], f32, tag=f"gates{c}")
                        for n in range(2):
                            sl = ds(n * 512, 512)
                            nc.tensor.matmul(gates[:, sl], hT[c][pp][0],
                                             whh_sb[c][0][:, sl],
                                             start=True, stop=False)
                            nc.tensor.matmul(gates[:, sl], hT[c][pp][1],
                                             whh_sb[c][1][:, sl],
                                             start=False, stop=False)
                            nc.tensor.matmul(gates[:, sl], ident8,
                                             gxch[c][:, k, sl],
                                             start=False, stop=True)
                        S = spl.tile([ROWS, G], f32, tag=f"S{c}")
                        nc.scalar.activation(S, gates, SIG)
                        Si, Sf = S[:, 0:H], S[:, H:2 * H]
                        Sg, So = S[:, 2 * H:3 * H], S[:, 3 * H:4 * H]
                        p_ = tmp.tile([ROWS, H], f32, tag=f"p{c}")
                        nc.vector.tensor_mul(p_, Si, Sg)
                        u = tmp.tile([ROWS, H], f32, tag=f"u{c}")
                        nc.vector.scalar_tensor_tensor(
                            out=u, in0=p_, scalar=2.0, in1=Si,
                            op0=MULT, op1=SUB)
                        v = tmp.tile([ROWS, H], f32, tag=f"v{c}")
                        nc.vector.tensor_mul(v, Sf, cst[c][pp])
                        cn = cst[c][1 - pp]
                        nc.vector.tensor_add(cn, u, v)
                        sc = tmp.tile([ROWS, H], f32, tag=f"sc{c}")
                        nc.scalar.activation(sc, cn, SIG, scale=2.0)
                        p2_ = tmp.tile([ROWS, H], f32, tag=f"q{c}")
                        nc.vector.tensor_mul(p2_, So, sc)
                        h = hpl.tile([ROWS, H], f32, tag=f"h{c}")
                        nc.vector.scalar_tensor_tensor(
                            out=h, in0=p2_, scalar=2.0, in1=So,
                            op0=MULT, op1=SUB)
                        hcur.append(h)
                        for kh in range(2):
                            pt = psT.tile([128, ROWS], f32, tag="pt")
                            nc.tensor.transpose(
                                pt, h[:, kh * 128:(kh + 1) * 128], ident8f)
                            nc.vector.tensor_copy(hT[c][1 - pp][kh], pt)
                    # combine: L_r = scan0 - scan1 (rows 0:4),
                    #          L_i = scan2 + scan3 (rows 4:8):
                    # st = hcur[1] * sgn + hcur[0], sgn = [-1]*4 + [1]*4
                    nc.vector.scalar_tensor_tensor(
                        out=st[:, k, :], in0=hcur[1], scalar=sgn_sb,
                        in1=hcur[0], op0=MULT, op1=mybir.AluOpType.add)
                nc.sync.dma_start(
                    out=hst[ds(iv, U), :, :].rearrange("u p h -> p u h"),
                    in_=st)

        # ---------------- phase 3: transpose to (b, h, t), interleave ----
        with ExitStack() as p3:
            p3c = p3.enter_context(tc.tile_pool(name="p3c", bufs=1))
            lp = p3.enter_context(tc.tile_pool(name="lp", bufs=4))
            tp = p3.enter_context(tc.tile_pool(name="tp", bufs=4, space="PSUM"))
            op = p3.enter_context(tc.tile_pool(name="op", bufs=4))

            id128 = p3c.tile([128, 128], f32, tag="id128")
            make_identity(nc, id128)

            for b in range(BL):
                for t in range(TC):
                    lr = lp.tile([128, H], f32, tag="lr")
                    li = lp.tile([128, H], f32, tag="li")
                    nc.sync.dma_start(
                        out=lr, in_=hst[t * 128:(t + 1) * 128, b, :])
                    nc.sync.dma_start(
                        out=li, in_=hst[t * 128:(t + 1) * 128, BL + b, :])
                    for hb in range(2):
                        hsl = ds(hb * 128, 128)
                        ptr = tp.tile([128, 128], f32, tag="ptr")
                        nc.tensor.transpose(ptr, lr[:, hsl], id128)
                        pti = tp.tile([128, 128], f32, tag="pti")
                        nc.tensor.transpose(pti, li[:, hsl], id128)
                        ot = op.tile([128, 256], f32, tag="ot")
                        otv = ot.rearrange("p (t two) -> p t two", two=2)
                        nc.vector.tensor_copy(otv[:, :, 0], ptr)
                        nc.vector.tensor_copy(otv[:, :, 1], pti)
                        nc.sync.dma_start(
                            out=out[b, hsl, ds(2 * t * 128, 256)], in_=ot)

    nc.compile()
    return nc


_CACHE = {}


def get_program(T):
    if T not in _CACHE:
        _CACHE[T] = build_program(T)
    return _CACHE[T]


def _pack_weights(Wih, Whh, bih, bhh):
    Wih = np.array(Wih, dtype=np.float32, copy=True)
    Whh = np.array(Whh, dtype=np.float32, copy=True)
    b = (np.asarray(bih) + np.asarray(bhh)).astype(np.float32)
    # pre-scale g gate (rows 2H:3H) by 2 so sigmoid(2g) gives tanh via 2s-1
    Wih[2 * H:3 * H] *= 2.0
    Whh[2 * H:3 * H] *= 2.0
    b[2 * H:3 * H] *= 2.0
    return np.ascontiguousarray(Wih.T), np.ascontiguousarray(Whh.T), b


def kernel(x_real, x_imag, Wih_r, Whh_r, bih_r, bhh_r,
           Wih_i, Whh_i, bih_i, bhh_i):
    x_real = np.asarray(x_real, dtype=np.float32)
    x_imag = np.asarray(x_imag, dtype=np.float32)
    T = x_real.shape[2]
    nc = get_program(T)

    wihT_r, whhT_r, b_r = _pack_weights(Wih_r, Whh_r, bih_r, bhh_r)
    wihT_i, whhT_i, b_i = _pack_weights(Wih_i, Whh_i, bih_i, bhh_i)
    wih_p = np.ascontiguousarray(np.stack([wihT_r, wihT_i]))
    whh_p = np.ascontiguousarray(np.stack([whhT_r, whhT_i]))
    bia_p = np.ascontiguousarray(np.stack([b_r, b_i]))
    sgn_p = np.array([[-1.0]] * BL + [[1.0]] * BL, dtype=np.float32)
    id8_p = np.eye(ROWS, dtype=np.float32)

    in_maps = []
    for c in range(NCORES):
        sl = slice(c * BL, (c + 1) * BL)
        in_maps.append({
            "xr": np.ascontiguousarray(x_real[sl]),
            "xi": np.ascontiguousarray(x_imag[sl]),
            "wih": wih_p, "whh": whh_p, "bias": bia_p,
            "sgn": sgn_p, "id8": id8_p,
        })
    res = run_bass_kernel_spmd(nc, in_maps, list(range(NCORES)))
    parts = []
    for c in range(NCORES):
        o = np.ascontiguousarray(res.results[c]["out"])  # [BL, H, 2T] f32
        parts.append(o.view(np.complex64))               # [BL, H, T]
    return np.concatenate(parts, axis=0)
